# revision 11
# baseline (speedup 1.0000x reference)
"""Trainium2 Bass kernel for a multi-agent DRQN (fc1 -> GRUCell -> fc2 + max/argmax).

Full-input contract: kernel(**inputs) takes the unsharded numpy inputs and
returns (nn_output [B,T,N,16], max_values [B,T,N], max_actions [B,T,N]).

Sharding: data-parallel over B across 8 NeuronCores (32 batch rows/core,
256 GRU lanes/core). Parameters replicated. The T=200 recurrence runs
locally per core with no collectives.

Device layout: "transposed" activations — features on SBUF partitions,
the 256 batch lanes on the free dimension.
  - obs is host-pretransposed to [T/8, 128, 8*256] so each DMA chunk is a
    contiguous 1MB transfer holding 8 timesteps of x^T.
  - fc1: a^T = relu(W1_obs @ x^T + W1_onehot @ onehot^T + b1), batched two
    steps per matmul (N=512) into PSUM; relu+b1 on ACT.
  - input-side gate projections gi = W_ih @ a^T are batched 2 steps ahead
    of the recurrence (they do not depend on h); the per-step recurrent
    matmuls gh = W_hh @ h accumulate into the same PSUM banks.
  - r,z are packed on 128 partitions -> one sigmoid per step.
  - n-gate: u = r*(hn + b_hhn) via fused scalar_tensor_tensor; v = u + gi_n;
    n = tanh(v + b_ihn) (bias folded into ACT).
  - h' = n + z*(h - n) on DVE.
  - Q head: q = [h;1] @ [W2.T; b2] row-major ([128 lanes, 16 actions]) into
    one PSUM bank per 16-step window; one ACT copy to SBUF per window, then
    max/argmax as free-dim DVE reductions using a mask * reverse-iota trick
    (exact first-index argmax).
"""

import sys

sys.path.insert(0, "/opt/trn_rl_repo")

import numpy as np

# Problem constants (hardcoded per the self-contained contract).
B, T, N_AGENTS = 256, 200, 8
OBS_DIM, OUT_DIM, RNN_IN, RNN_H = 128, 16, 64, 64
N_CORES = 8
B_SH = B // N_CORES            # 32 batch rows per core
R = B_SH * N_AGENTS            # 256 GRU lanes per core
DMA_T = 8                      # timesteps per obs DMA chunk
CT = 2                         # timesteps per compute chunk (PSUM batching)
WIN = 16                       # timesteps per output/argmax window (1 PSUM bank)

_RUNNER_CACHE = {}
_NC_CACHE = {}


def _windows(t_total):
    out, t = [], 0
    while t < t_total:
        w = min(WIN, t_total - t)
        out.append((t, w))
        t += w
    return out


def _build_nc(t_total=T, use_f32r=True):
    """Build the Bass program for one core (SPMD: all cores identical)."""
    import concourse.bacc as bacc
    import concourse.mybir as mybir
    import concourse.tile as tile

    assert t_total % DMA_T == 0 and WIN % CT == 0
    f32 = mybir.dt.float32
    f32r = mybir.dt.float32r

    nc = bacc.Bacc("TRN2", target_bir_lowering=False, debug=False,
                   num_devices=N_CORES)

    xT = nc.dram_tensor("xT", [t_total // DMA_T, 128, DMA_T * R], f32,
                        kind="ExternalInput")
    w1obsT = nc.dram_tensor("w1obsT", [128, RNN_IN], f32, kind="ExternalInput")
    w1ohT = nc.dram_tensor("w1ohT", [N_AGENTS, RNN_IN], f32, kind="ExternalInput")
    onehot2 = nc.dram_tensor("onehot2", [N_AGENTS, CT * R], f32, kind="ExternalInput")
    wihrzT = nc.dram_tensor("wihrzT", [RNN_IN, 128], f32, kind="ExternalInput")
    wihnT = nc.dram_tensor("wihnT", [RNN_IN, RNN_H], f32, kind="ExternalInput")
    whhrzT = nc.dram_tensor("whhrzT", [RNN_H, 128], f32, kind="ExternalInput")
    whhnT = nc.dram_tensor("whhnT", [RNN_H, RNN_H], f32, kind="ExternalInput")
    w2T = nc.dram_tensor("w2T", [RNN_H + 1, OUT_DIM], f32, kind="ExternalInput")
    bias_rz = nc.dram_tensor("bias_rz", [128, 1], f32, kind="ExternalInput")
    b1v = nc.dram_tensor("b1v", [RNN_IN, 1], f32, kind="ExternalInput")
    bihn = nc.dram_tensor("bihn", [RNN_H, 1], f32, kind="ExternalInput")
    bhhn = nc.dram_tensor("bhhn", [RNN_H, 1], f32, kind="ExternalInput")
    riota = nc.dram_tensor("riota", [128, OUT_DIM], f32, kind="ExternalInput")

    qout = nc.dram_tensor("qout", [128, t_total * 2 * OUT_DIM], f32,
                          kind="ExternalOutput")
    maxv = nc.dram_tensor("maxv", [128, t_total * 2], f32, kind="ExternalOutput")
    maxa = nc.dram_tensor("maxa", [128, t_total * 2], f32, kind="ExternalOutput")

    A = mybir.AluOpType
    AF = mybir.ActivationFunctionType
    AX = mybir.AxisListType

    def mmcast(ap):
        return ap.bitcast(f32r) if use_f32r else ap

    with tile.TileContext(nc) as tc:
        with (
            tc.tile_pool(name="consts", bufs=1) as cpool,
            tc.tile_pool(name="state", bufs=1) as spool,
            tc.tile_pool(name="xbuf", bufs=2) as xpool,
            tc.tile_pool(name="abuf", bufs=2) as apool,
            tc.tile_pool(name="sbwork", bufs=3) as wpool,
            tc.tile_pool(name="qsb", bufs=2) as qspool,
            tc.tile_pool(name="redux", bufs=2) as rpool,
            tc.tile_pool(name="ps_fc1", bufs=2, space="PSUM") as pfc1,
            tc.tile_pool(name="ps_rz", bufs=2, space="PSUM") as prz,
            tc.tile_pool(name="ps_ih", bufs=2, space="PSUM") as pih,
            tc.tile_pool(name="ps_q", bufs=2, space="PSUM") as pq,
        ):
            # --- constants into SBUF ---
            c_w1obsT = cpool.tile([128, RNN_IN], f32)
            c_w1ohT = cpool.tile([N_AGENTS, RNN_IN], f32)
            c_onehot2 = cpool.tile([N_AGENTS, CT * R], f32)
            c_wihrzT = cpool.tile([RNN_IN, 128], f32)
            c_wihnT = cpool.tile([RNN_IN, RNN_H], f32)
            c_whhrzT = cpool.tile([RNN_H, 128], f32)
            c_whhnT = cpool.tile([RNN_H, RNN_H], f32)
            c_w2T = cpool.tile([RNN_H + 1, OUT_DIM], f32)
            c_bias_rz = cpool.tile([128, 1], f32)
            c_b1v = cpool.tile([RNN_IN, 1], f32)
            c_bihn = cpool.tile([RNN_H, 1], f32)
            # b_hhn lives at base partition 64 so the fused STT's operands
            # (hn in PSUM partitions 64:128, r in SBUF partitions 64:128)
            # share a base partition, as the HW BIR verifier requires.
            c_bhhn = cpool.tile([128, 1], f32)
            c_riota = cpool.tile([128, OUT_DIM], f32)
            for t_, d_ in [(c_w1obsT, w1obsT), (c_w1ohT, w1ohT),
                           (c_onehot2, onehot2), (c_wihrzT, wihrzT),
                           (c_wihnT, wihnT), (c_whhrzT, whhrzT),
                           (c_whhnT, whhnT), (c_w2T, w2T),
                           (c_bias_rz, bias_rz), (c_b1v, b1v),
                           (c_bihn, bihn), (c_riota, riota)]:
                nc.sync.dma_start(out=t_[:], in_=d_[:])
            nc.sync.dma_start(out=c_bhhn[RNN_H:128, :], in_=bhhn[:])

            # --- recurrent state: rows 0:64 = h, row 64 = ones (for q bias) ---
            h_state = spool.tile([RNN_H + 1, R], f32)
            nc.vector.memset(h_state[0:RNN_H, :], 0.0)
            nc.vector.memset(h_state[RNN_H:RNN_H + 1, :], 1.0)

            x_tile = None
            for t_start, wlen in _windows(t_total):
                psum_q = pq.tile([128, WIN * 2 * OUT_DIM], f32, tag="q")
                for pair in range(wlen // CT):
                    t0 = t_start + pair * CT
                    d = t0 // DMA_T
                    if t0 % DMA_T == 0:
                        x_tile = xpool.tile([128, DMA_T * R], f32)
                        nc.sync.dma_start(out=x_tile[:], in_=xT[d])
                    xa = x_tile[:, (t0 % DMA_T) * R:(t0 % DMA_T) * R + CT * R]

                    # fc1 (2 steps batched): psum_fc1 = W1obs @ x^T + W1oh @ oh^T
                    psum_fc1 = pfc1.tile([RNN_IN, CT * R], f32)
                    nc.tensor.matmul(psum_fc1[:], mmcast(c_w1obsT[:]), mmcast(xa),
                                     start=True, stop=False)
                    nc.tensor.matmul(psum_fc1[:], c_w1ohT[:], c_onehot2[:],
                                     start=False, stop=True)
                    a_sb = apool.tile([RNN_IN, CT * R], f32)
                    nc.scalar.activation(a_sb[:], psum_fc1[:], AF.Relu,
                                         bias=c_b1v[:])

                    # Per-step PSUM tiles: each accumulation group must open
                    # and close within its own 2KB bank zero-region before
                    # any engine reads it.
                    for k in range(CT):
                        cs = slice(k * R, (k + 1) * R)
                        # input-side projections for this step (h-independent)
                        psum_rz = prz.tile([128, R], f32, tag="przs")
                        nc.tensor.matmul(psum_rz[:], mmcast(c_wihrzT[:]),
                                         mmcast(a_sb[:, cs]),
                                         start=True, stop=False)
                        psum_ih = pih.tile([128, R], f32, tag="pihs")
                        nc.tensor.matmul(psum_ih[0:RNN_H, :], mmcast(c_wihnT[:]),
                                         mmcast(a_sb[:, cs]),
                                         start=True, stop=True)
                        # recurrent projections for this step
                        nc.tensor.matmul(psum_rz[:], mmcast(c_whhrzT[:]),
                                         mmcast(h_state[0:RNN_H, :]),
                                         start=False, stop=True)
                        nc.tensor.matmul(psum_ih[RNN_H:128, :],
                                         mmcast(c_whhnT[:]),
                                         mmcast(h_state[0:RNN_H, :]),
                                         start=True, stop=True)
                        # z,r = sigmoid(gzr + bzr)  [128 x 256]
                        # (z packed at partitions 0:64, r at 64:128 so every
                        #  later two-SBUF-operand DVE op has equal bases)
                        rz = wpool.tile([128, R], f32, tag="rz")
                        nc.scalar.activation(rz[:], psum_rz[:], AF.Sigmoid,
                                             bias=c_bias_rz[:])
                        # u = r * (hn + b_hhn)
                        u = wpool.tile([RNN_H, R], f32, tag="u")
                        nc.vector.scalar_tensor_tensor(
                            u[:], psum_ih[RNN_H:128, :], c_bhhn[RNN_H:128, :],
                            rz[RNN_H:128, :], op0=A.add, op1=A.mult)
                        # v = u + gi_n
                        v = wpool.tile([RNN_H, R], f32, tag="v")
                        nc.vector.tensor_add(v[:], u[:], psum_ih[0:RNN_H, :])
                        # n = tanh(v + b_ihn)
                        n_t = wpool.tile([RNN_H, R], f32, tag="n")
                        nc.scalar.activation(n_t[:], v[:], AF.Tanh, bias=c_bihn[:])
                        # h' = n + z*(h - n)
                        w_t = wpool.tile([RNN_H, R], f32, tag="w")
                        nc.vector.tensor_sub(w_t[:], h_state[0:RNN_H, :], n_t[:])
                        x_t = wpool.tile([RNN_H, R], f32, tag="x")
                        nc.vector.tensor_mul(x_t[:], rz[0:RNN_H, :], w_t[:])
                        nc.vector.tensor_add(h_state[0:RNN_H, :], n_t[:], x_t[:])
                        # q = [h;1] @ [W2.T; b2]  (row-major, two 128-lane halves)
                        qc = (pair * CT + k) * 2 * OUT_DIM
                        for half in range(2):
                            first = pair == 0 and k == 0 and half == 0
                            last = (pair == wlen // CT - 1 and k == CT - 1
                                    and half == 1)
                            nc.tensor.matmul(
                                psum_q[:, qc + half * OUT_DIM:
                                       qc + (half + 1) * OUT_DIM],
                                h_state[:, half * 128:(half + 1) * 128],
                                c_w2T[:], start=first, stop=last)

                # ---- window tail: copy q to SBUF; max + first-index argmax ----
                ng = wlen * 2
                q_sb = qspool.tile([128, WIN * 2 * OUT_DIM], f32, tag="qsb")
                nc.scalar.copy(q_sb[:, :ng * OUT_DIM], psum_q[:, :ng * OUT_DIM])
                qv = q_sb[:, :ng * OUT_DIM].rearrange("p (g a) -> p g a", a=OUT_DIM)
                m_t = rpool.tile([128, WIN * 2], f32, tag="m")
                nc.vector.tensor_reduce(m_t[:, :ng], qv, AX.X, A.max)
                mask = rpool.tile([128, WIN * 2 * OUT_DIM], f32, tag="mask")
                maskv = mask[:, :ng * OUT_DIM].rearrange("p (g a) -> p g a",
                                                         a=OUT_DIM)
                nc.vector.tensor_tensor(
                    maskv, qv,
                    m_t[:, :ng].unsqueeze(2).broadcast_to([128, ng, OUT_DIM]),
                    op=A.is_ge)
                nc.vector.tensor_tensor(
                    maskv, maskv,
                    c_riota[:].unsqueeze(1).broadcast_to([128, ng, OUT_DIM]),
                    op=A.mult)
                rm = rpool.tile([128, WIN * 2], f32, tag="rm")
                nc.vector.tensor_reduce(rm[:, :ng], maskv, AX.X, A.max)
                am = rpool.tile([128, WIN * 2], f32, tag="am")
                nc.vector.tensor_scalar(am[:, :ng], rm[:, :ng], -1.0,
                                        float(OUT_DIM), op0=A.mult, op1=A.add)
                c0 = t_start * 2
                nc.sync.dma_start(out=qout[:, c0 * OUT_DIM:(c0 + ng) * OUT_DIM],
                                  in_=q_sb[:, :ng * OUT_DIM])
                nc.sync.dma_start(out=maxv[:, c0:c0 + ng], in_=m_t[:, :ng])
                nc.sync.dma_start(out=maxa[:, c0:c0 + ng], in_=am[:, :ng])

    nc.compile()
    return nc


def _host_prep(inputs, t_total=T):
    """Split/transform the full inputs into per-core in_maps."""
    obs = np.ascontiguousarray(inputs["obs_history"], dtype=np.float32)
    agent = np.asarray(inputs["agent"])
    W1 = np.asarray(inputs["W1"], dtype=np.float32)
    b1 = np.asarray(inputs["b1"], dtype=np.float32)
    W_ih = np.asarray(inputs["W_ih"], dtype=np.float32)
    W_hh = np.asarray(inputs["W_hh"], dtype=np.float32)
    b_ih = np.asarray(inputs["b_ih"], dtype=np.float32)
    b_hh = np.asarray(inputs["b_hh"], dtype=np.float32)
    W2 = np.asarray(inputs["W2"], dtype=np.float32)
    b2 = np.asarray(inputs["b2"], dtype=np.float32)

    agent_ids = np.max(agent[0, :, :, 0], axis=0).astype(np.int64)  # [N]
    oh = np.zeros((N_AGENTS, N_AGENTS), np.float32)
    oh[np.arange(N_AGENTS), agent_ids] = 1.0

    # onehot2[j, c] for c = (k, b, n) flattened: slot n = c % 8
    onehot2 = np.ascontiguousarray(
        np.tile(oh.T[:, None, :], (1, CT * B_SH, 1)).reshape(N_AGENTS, CT * R))

    # gate packing on partitions: z at 0:64, r at 64:128 (see kernel note)
    zr = np.r_[RNN_H:2 * RNN_H, 0:RNN_H]
    shared = {
        "w1obsT": np.ascontiguousarray(W1[:, :OBS_DIM].T),
        "w1ohT": np.ascontiguousarray(W1[:, OBS_DIM:].T),
        "onehot2": onehot2,
        "wihrzT": np.ascontiguousarray(W_ih[zr, :].T),
        "wihnT": np.ascontiguousarray(W_ih[2 * RNN_H:, :].T),
        "whhrzT": np.ascontiguousarray(W_hh[zr, :].T),
        "whhnT": np.ascontiguousarray(W_hh[2 * RNN_H:, :].T),
        "w2T": np.ascontiguousarray(np.vstack([W2.T, b2[None, :]])),
        "bias_rz": np.ascontiguousarray(
            (b_ih[zr] + b_hh[zr])[:, None]),
        "b1v": np.ascontiguousarray(b1[:, None]),
        "bihn": np.ascontiguousarray(b_ih[2 * RNN_H:][:, None]),
        "bhhn": np.ascontiguousarray(b_hh[2 * RNN_H:][:, None]),
        "riota": np.tile((OUT_DIM - np.arange(OUT_DIM, dtype=np.float32))[None, :],
                         (128, 1)),
    }

    in_maps = []
    for c in range(N_CORES):
        sh = obs[c * B_SH:(c + 1) * B_SH, :t_total]       # [32, T, 8, 128]
        xt = sh.transpose(1, 3, 0, 2).reshape(t_total, OBS_DIM, R)
        xt = (xt.reshape(t_total // DMA_T, DMA_T, OBS_DIM, R)
                .transpose(0, 2, 1, 3)
                .reshape(t_total // DMA_T, OBS_DIM, DMA_T * R))
        in_maps.append({"xT": np.ascontiguousarray(xt), **shared})
    return in_maps


def _assemble(core_outs, t_total=T):
    """Per-core device outputs -> full (nn_output, max_values, max_actions)."""
    nn, mv, ma = [], [], []
    for res in core_outs:
        # qout cols = t*32 + half*16 + a ; rows p -> lane = half*128 + p
        q = res["qout"].reshape(128, t_total, 2, OUT_DIM)
        q = q.transpose(1, 2, 0, 3).reshape(t_total, R, OUT_DIM)
        nn.append(q.reshape(t_total, B_SH, N_AGENTS, OUT_DIM).transpose(1, 0, 2, 3))
        for src, dst in ((res["maxv"], mv), (res["maxa"], ma)):
            r_ = src.reshape(128, t_total, 2).transpose(1, 2, 0).reshape(t_total, R)
            dst.append(r_.reshape(t_total, B_SH, N_AGENTS).transpose(1, 0, 2))
    return (np.concatenate(nn, axis=0), np.concatenate(mv, axis=0),
            np.concatenate(ma, axis=0))


def _make_runner(nc):
    """Build a cached jitted SPMD executor for the prebuilt Bass program."""
    import jax
    from jax.sharding import Mesh, PartitionSpec, NamedSharding
    from jax.experimental.shard_map import shard_map
    import concourse.mybir as mybir
    from concourse.bass2jax import (_bass_exec_p, install_neuronx_cc_hook,
                                    partition_id_tensor)

    install_neuronx_cc_hook()
    partition_name = (nc.partition_id_tensor.name
                      if nc.partition_id_tensor else None)
    in_names, out_names, out_avals = [], [], []
    for alloc in nc.m.functions[0].allocations:
        if not isinstance(alloc, mybir.MemoryLocationSet):
            continue
        name = alloc.memorylocations[0].name
        if alloc.kind == "ExternalInput":
            if name != partition_name:
                in_names.append(name)
        elif alloc.kind == "ExternalOutput":
            out_names.append(name)
            out_avals.append(jax.core.ShapedArray(
                tuple(alloc.tensor_shape), mybir.dt.np(alloc.dtype)))
    n_params = len(in_names)
    all_names = in_names + out_names + ([partition_name] if partition_name else [])

    def _body(*args):
        operands = list(args)
        if partition_name is not None:
            operands.append(partition_id_tensor())
        outs = _bass_exec_p.bind(
            *operands, out_avals=tuple(out_avals), in_names=tuple(all_names),
            out_names=tuple(out_names), lowering_input_output_aliases=(),
            sim_require_finite=True, sim_require_nnan=True, nc=nc)
        return tuple(outs)

    devices = jax.devices()[:N_CORES]
    mesh = Mesh(np.asarray(devices), ("core",))
    n_outs = len(out_names)
    fn = jax.jit(
        shard_map(_body, mesh=mesh,
                  in_specs=(PartitionSpec("core"),) * (n_params + n_outs),
                  out_specs=(PartitionSpec("core"),) * n_outs,
                  check_rep=False),
        donate_argnums=tuple(range(n_params, n_params + n_outs)),
        keep_unused=True)
    sharding = NamedSharding(mesh, PartitionSpec("core"))
    return fn, in_names, out_names, out_avals, sharding


def run_on_hw(in_maps, nc):
    """Execute the SPMD program; returns list of per-core output dicts."""
    import jax
    key = id(nc)
    if key not in _RUNNER_CACHE:
        _RUNNER_CACHE[key] = _make_runner(nc)
    fn, in_names, out_names, out_avals, sharding = _RUNNER_CACHE[key]
    concat_in = [
        jax.device_put(
            np.concatenate([np.asarray(m[nm]) for m in in_maps], axis=0),
            sharding)
        for nm in in_names]
    zeros = [
        jax.device_put(
            np.zeros((N_CORES * a.shape[0], *a.shape[1:]), a.dtype), sharding)
        for a in out_avals]
    outs = fn(*concat_in, *zeros)
    outs = [np.asarray(o) for o in outs]
    return [
        {nm: outs[i].reshape(N_CORES, *out_avals[i].shape)[c]
         for i, nm in enumerate(out_names)}
        for c in range(N_CORES)]


def get_nc(t_total=T, use_f32r=True):
    key = (t_total, use_f32r)
    if key not in _NC_CACHE:
        _NC_CACHE[key] = _build_nc(t_total, use_f32r)
    return _NC_CACHE[key]


def kernel(**inputs):
    nc = get_nc()
    in_maps = _host_prep(inputs)
    core_outs = run_on_hw(in_maps, nc)
    return _assemble(core_outs)
